# revision 23
# baseline (speedup 1.0000x reference)
"""Trainium2 Bass kernel for nn_AttentionLayer (pooled attention).

Reference computation (per batch b, step t):
    q = x @ Wq + bq                          # (N, D)
    k = mean-pool-8(x) @ Wk + bk             # (C, D)   [pool commutes with linear]
    v = mean-pool-8(x) @ Wv + bv             # (C, D)
    per head h (HD=64):
        score = qh @ khT / 8 + adp_pos       # (N, C)
        attn  = softmax(score, axis=-1)
        outh  = attn @ vh                    # (N, HD)
    y = concat(outh) @ Wo + bo               # (N, D)

Sharding: data-parallel over batch B=16 -> 2 per NeuronCore x 8 cores.
All matmuls in bf16 (f32 PSUM accumulation).

v2 design (vs v1): the softmax runs entirely in the TRANSPOSED
orientation, eliminating the per-slice attention DRAM round-trip +
xbar transpose that starved the PE in v1:
  - scores are computed directly as scT[c, n] (lhsT = kpT head slice,
    rhs = qT head slice, K=64).
  - u = exp(scT/8) * exp(adp)^T; softmax denominators come from
    matmuls with one-hot column tiles (e_all) accumulating all 8 heads
    into rows 0..7 of one PSUM tile; one reciprocal_approx_fast.
  - normalization is DEFERRED: o_unnorm^T = vp^T-slices @ u_T with the
    head pair packed into one PSUM tile via tile_position (0,0)/(0,64);
    the evacuation multiplies by r broadcast tiles (r spilled to DRAM
    and re-loaded with partition-replicating cast DMAs).
  - pooling is a strided DVE reduce over xT (no PE work).
  - bo is added during the y evacuation (DVE tensor_tensor add).
"""

import os

import numpy as np

B, T, N, D = 16, 12, 1024, 512
H, HD, C = 8, 64, 128
NCORES = 8
BS = B // NCORES          # batch per core
NBT = BS * T              # (b, t) slices per core
MT = N // 128             # m-tiles per (b, t) slice  = 8
CI = D // 128             # contraction chunks        = 4
POOL = N // C             # pooling factor            = 8


def build_kernel(nc, n_bt=NBT):
    """Emit the full per-core kernel graph into `nc` (a bacc.Bacc)."""
    import concourse.bass as bass
    import concourse.tile as tile
    from concourse import mybir

    f32 = mybir.dt.float32
    bf16 = mybir.dt.bfloat16
    fp8 = mybir.dt.float8e4
    AF = mybir.ActivationFunctionType
    ALU = mybir.AluOpType

    M = n_bt * N

    x_in = nc.dram_tensor("x", [BS, T, N, D], f32, kind="ExternalInput").ap()
    Wq_in = nc.dram_tensor("Wq", [D, D], f32, kind="ExternalInput").ap()
    bq_in = nc.dram_tensor("bq", [D], f32, kind="ExternalInput").ap()
    Wk_in = nc.dram_tensor("Wk", [D, D], f32, kind="ExternalInput").ap()
    bk_in = nc.dram_tensor("bk", [D], f32, kind="ExternalInput").ap()
    Wv_in = nc.dram_tensor("Wv", [D, D], f32, kind="ExternalInput").ap()
    bv_in = nc.dram_tensor("bv", [D], f32, kind="ExternalInput").ap()
    Wo_in = nc.dram_tensor("Wo", [D, D], f32, kind="ExternalInput").ap()
    bo_in = nc.dram_tensor("bo", [D], f32, kind="ExternalInput").ap()
    adp_in = nc.dram_tensor("adp_pos", [N, C], f32, kind="ExternalInput").ap()
    y_out = nc.dram_tensor("out", [BS, T, N, D], f32, kind="ExternalOutput").ap()

    x_flat = x_in.rearrange("b t n d -> (b t n) d")
    y_flat = y_out.rearrange("b t n d -> (b t n) d")

    with tile.TileContext(nc) as tc:
        with (
            tc.tile_pool(name="const", bufs=1) as const_pool,
            tc.tile_pool(name="dram", bufs=1, space="DRAM") as dram_pool,
            tc.tile_pool(name="xt", bufs=3) as xt_pool,
            tc.tile_pool(name="qt", bufs=2) as qt_pool,
            tc.tile_pool(name="pooled", bufs=2) as pooled_pool,
            tc.tile_pool(name="smx", bufs=2) as smx_pool,
            tc.tile_pool(name="outt", bufs=2) as outt_pool,
            tc.tile_pool(name="ysb", bufs=2) as y_pool,
            tc.tile_pool(name="psA", bufs=2, space="PSUM") as psA,
            tc.tile_pool(name="psB", bufs=2, space="PSUM") as psB,
        ):
            # bf16 copy of x in DRAM feeding the xbar transpose loads.
            # The first two slices' casts are issued before the weight
            # loads so the slice-0 transposes can start ASAP.
            x16 = dram_pool.tile([M, D], bf16, name="x16")

            # ---------------- constants / weights preload ----------------
            # W* layout: [128, ci*512 + dout] = W[ci*128 + p, dout]  (bf16)
            # Interleaved with the first two slices' x casts so the
            # critical prologue chain (x16 -> xT -> q-proj) starts ASAP.
            w_sb = {}

            def load_w(nm, w_ap):
                w_t = const_pool.tile([128, CI * D], bf16, name=f"W{nm}_sb")
                nc.gpsimd.dma_start(
                    out=w_t[:].rearrange("p (ci dout) -> p ci dout", ci=CI),
                    in_=w_ap.rearrange("(ci p) dout -> p ci dout", p=128),
                )
                w_sb[nm] = w_t

            for ci in range(CI):
                nc.gpsimd.dma_start(
                    out=x16[0:N, ci * 128:(ci + 1) * 128],
                    in_=x_flat[0:N, ci * 128:(ci + 1) * 128],
                )
            # Wq in fp8e4 (DoubleRow): load f32, scale x16 (avoids
            # e4m3 subnormals for the 0.02-scale weights; undone at the
            # qT evac), cast to fp8 via sbuf->sbuf cast DMA.
            wq_f = const_pool.tile([128, CI * D], f32, name="wq_f")
            nc.gpsimd.dma_start(
                out=wq_f[:].rearrange("p (ci dout) -> p ci dout", ci=CI),
                in_=Wq_in.rearrange("(ci p) dout -> p ci dout", p=128),
            )
            wq_s = const_pool.tile([128, CI * D], bf16, name="wq_s")
            nc.scalar.activation(wq_s[:], wq_f[:], AF.Copy, scale=16.0)
            w_q8 = const_pool.tile([128, CI * D], fp8, name="w_q8")
            nc.gpsimd.dma_start(out=w_q8[:], in_=wq_s[:])
            if n_bt > 1:
                for ci in range(CI):
                    nc.gpsimd.dma_start(
                        out=x16[N:2 * N, ci * 128:(ci + 1) * 128],
                        in_=x_flat[N:2 * N, ci * 128:(ci + 1) * 128],
                    )
            load_w("k", Wk_in)
            load_w("v", Wv_in)
            load_w("o", Wo_in)

            # per-partition bias tiles [128, dt] for the transposed q/k evacs
            bq_sb = const_pool.tile([128, CI], f32, name="bq_sb")
            nc.sync.dma_start(out=bq_sb[:], in_=bq_in.rearrange("(dt p) -> p dt", p=128))
            bk_sb = const_pool.tile([128, CI], f32, name="bk_sb")
            nc.sync.dma_start(out=bk_sb[:], in_=bk_in.rearrange("(dt p) -> p dt", p=128))

            # bv*8 row (vp psum holds 8*(xp_sum/8 @ Wv + bv); evac scales 1/8)
            bv_row = const_pool.tile([1, D], f32, name="bv_row")
            nc.sync.dma_start(out=bv_row[:], in_=bv_in.unsqueeze(0))
            bv8_row = const_pool.tile([1, D], bf16, name="bv8_row")
            nc.scalar.activation(bv8_row[:], bv_row[:], AF.Copy, scale=float(POOL))

            bo_row = const_pool.tile([1, D], f32, name="bo_row")
            nc.sync.dma_start(out=bo_row[:], in_=bo_in.unsqueeze(0))

            ones_k1 = const_pool.tile([1, 128], bf16, name="ones_k1")
            nc.vector.memset(ones_k1[:], 1.0)
            ones_f32 = const_pool.tile([1, 128], f32, name="ones_f32")
            nc.vector.memset(ones_f32[:], 1.0)

            # bo broadcast [128, 2*D] (two copies side by side, for the
            # two-mt y evacuation adds)
            bo_bc2 = const_pool.tile([128, 2 * D], f32, name="bo_bc2")
            ps_bo = psA.tile([128, 2 * D], f32, name="ps_bo", tag="A", bufs=2)
            for half in range(2):
                nc.tensor.matmul(
                    ps_bo[:, half * D:(half + 1) * D], ones_f32[:], bo_row[:],
                    start=True, stop=True,
                )
            nc.vector.tensor_copy(bo_bc2[:], ps_bo[:])

            # e_all: [128, 8*H]; slice h = cols [8h, 8h+8), ones in col h.
            # sums matmul lhsT -> head h's denominator lands in psum row h.
            e_all = const_pool.tile([128, 8 * H], bf16, name="e_all")
            nc.vector.memset(e_all[:], 0.0)
            for h in range(H):
                nc.vector.memset(e_all[:, 8 * h + h: 8 * h + h + 1], 1.0)

            # eadpT [c=128, n=N] = exp(adp_pos)^T  (bf16), built via natural
            # exp -> DRAM spill -> one xbar transpose.
            adp_f = const_pool.tile([128, MT * C], f32, name="adp_f")
            nc.sync.dma_start(
                out=adp_f[:].rearrange("p (nt c) -> p nt c", nt=MT),
                in_=adp_in.rearrange("(nt p) c -> p nt c", p=128),
            )
            eadp_nat = const_pool.tile([128, MT * C], bf16, name="eadp_nat")
            nc.scalar.activation(eadp_nat[:], adp_f[:], AF.Exp)
            eadp_dr = dram_pool.tile([N, C], bf16, name="eadp_dr")
            nc.sync.dma_start(
                out=eadp_dr[:].rearrange("(nt p) c -> p nt c", p=128),
                in_=eadp_nat[:].rearrange("p (nt c) -> p nt c", nt=MT),
            )
            eadpT = const_pool.tile([128, N], bf16, name="eadpT")
            nc.sync.dma_start(out=eadpT[:], in_=eadp_dr[:], transpose=True)

            # ---------------- per-slice emission helpers ----------------

            def stage_inputs(bt, cast=True):
                """x16 <- cast DMA (DRAM->DRAM, per column chunk so each
                xbar transpose can start as soon as its chunk lands);
                xT <- 4 xbar transposes."""
                r0 = bt * N
                xT = xt_pool.tile([128, CI * N], bf16, name="xT", tag="xT")
                xT8 = xt_pool.tile([128, CI * N], fp8, name="xT8", tag="xT8")
                for ci in range(CI):
                    if cast:
                        nc.gpsimd.dma_start(
                            out=x16[r0:r0 + N, ci * 128:(ci + 1) * 128],
                            in_=x_flat[r0:r0 + N, ci * 128:(ci + 1) * 128],
                        )
                    nc.sync.dma_start(
                        out=xT[:, ci * N:(ci + 1) * N],
                        in_=x16[r0:r0 + N, ci * 128:(ci + 1) * 128],
                        transpose=True,
                    )
                    nc.gpsimd.dma_start(
                        out=xT8[:, ci * N:(ci + 1) * N],
                        in_=xT[:, ci * N:(ci + 1) * N],
                    )
                return xT, xT8

            def pool_reduce(xTpair):
                """Strided DVE pooling over xT -> xpT. Emitted at the START
                of the previous slice's head loop so these DVE ops precede
                that slice's u-mults/reciprocal in the DVE queue."""
                xT, _ = xTpair
                xpT = pooled_pool.tile([128, CI * C], bf16, name="xpT")
                with nc.allow_low_precision("pool sums, bf16 ok"):
                    for ci in range(CI):
                        nc.vector.reduce_sum(
                            xpT[:, ci * C:(ci + 1) * C],
                            xT[:, ci * N:(ci + 1) * N]
                            .rearrange("p (j i) -> p j i", i=POOL),
                            axis=mybir.AxisListType.X,
                        )
                return xpT

            def proj_chunks(xTpair, xpT):
                """Generator of PE work for one slice's projections; yields
                between chunks so the caller interleaves it into the softmax
                head loop. Final yield returns (qT, kpT, vp)."""
                xT, xT8 = xTpair
                w_q8_v = w_q8[:].rearrange("p (ci dout) -> p ci dout", ci=CI)
                xT8_v = xT8[:].rearrange("p (ci n) -> p ci n", ci=CI)
                qT = qt_pool.tile([128, CI * N], bf16, name="qT")
                for dt in range(CI):
                    ps_q = psA.tile([128, N], f32, name="ps_q", tag="A", bufs=2)
                    for ci2 in range(0, CI, 2):
                        for mc in range(2):
                            nc.tensor.matmul(
                                ps_q[:, mc * 512:(mc + 1) * 512],
                                w_q8_v[:, ci2:ci2 + 2, dt * 128:(dt + 1) * 128],
                                xT8_v[:, ci2:ci2 + 2, mc * 512:(mc + 1) * 512],
                                start=(ci2 == 0),
                                stop=(ci2 == CI - 2),
                                perf_mode=mybir.MatmulPerfMode.DoubleRow,
                            )
                    nc.scalar.activation(
                        qT[:, dt * N:(dt + 1) * N], ps_q[:],
                        AF.Identity, scale=1.0 / 16.0, bias=bq_sb[:, dt:dt + 1],
                    )
                    yield None

                # kp (transposed) + vp (natural) share one psum tile
                ps_kv = psB.tile([128, N], f32, name="ps_kv", tag="B", bufs=2)
                kpT = pooled_pool.tile([128, CI * C], bf16, name="kpT")
                vp = pooled_pool.tile([128, D], bf16, name="vp")
                for dt in range(CI):
                    for ci in range(CI):
                        nc.tensor.matmul(
                            ps_kv[:, dt * 128:(dt + 1) * 128],
                            w_sb["k"][:, ci * D + dt * 128: ci * D + dt * 128 + 128],
                            xpT[:, ci * C:(ci + 1) * C],
                            start=(ci == 0),
                            stop=(ci == CI - 1),
                        )
                yield None
                for ci in range(CI):
                    nc.tensor.matmul(
                        ps_kv[:, 512:],
                        xpT[:, ci * C:(ci + 1) * C],
                        w_sb["v"][:, ci * D:(ci + 1) * D],
                        start=(ci == 0),
                        stop=False,
                    )
                nc.tensor.matmul(
                    ps_kv[:, 512:], ones_k1[:], bv8_row[:], start=False, stop=True
                )
                for dt in range(CI):
                    nc.scalar.activation(
                        kpT[:, dt * 128:(dt + 1) * 128],
                        ps_kv[:, dt * 128:(dt + 1) * 128],
                        AF.Identity, scale=1.0 / POOL, bias=bk_sb[:, dt:dt + 1],
                    )
                nc.scalar.activation(vp[:], ps_kv[:, 512:], AF.Copy, scale=1.0 / POOL)
                yield (qT, kpT, vp)

            def head_loop(qkv, proj_gen):
                """Scores/exp/u/sums for all 8 heads of one slice, with the
                NEXT slice's q-projection chunks interleaved into the PE
                gaps. Returns (u_all, ps_s, next_qT)."""
                qT, kpT, vp = qkv
                u_all = smx_pool.tile([128, H * N], bf16, name="u_all", bufs=1)
                ps_s = psB.tile([128, N], f32, name="ps_s", tag="B", bufs=2)
                next_qkv = None
                pending_sums = []

                def drain_one_proj():
                    nonlocal next_qkv
                    if proj_gen is not None:
                        try:
                            r = next(proj_gen)
                            if r is not None:
                                next_qkv = r
                        except StopIteration:
                            pass

                for h in range(H):
                    ph = (h % 2) * 64
                    dt = h // 2
                    ps_sc = psA.tile([128, N], f32, name="ps_sc", tag="A", bufs=2)
                    for half in range(2):
                        nc.tensor.matmul(
                            ps_sc[:, half * 512:(half + 1) * 512],
                            kpT[ph:ph + 64, dt * 128:(dt + 1) * 128],
                            qT[ph:ph + 64, dt * N + half * 512: dt * N + (half + 1) * 512],
                            start=True, stop=True,
                        )
                    exp_sb = smx_pool.tile([128, N], bf16, name="exp_sb", bufs=2)
                    nc.scalar.activation(exp_sb[:], ps_sc[:], AF.Exp, scale=1.0 / 8.0)
                    nc.vector.tensor_tensor(
                        u_all[:, h * N:(h + 1) * N], exp_sb[:], eadpT[:], op=ALU.mult
                    )
                    # interleave projection work into the exp/u latency gap
                    drain_one_proj()
                    if pending_sums:
                        hp = pending_sums.pop(0)
                        for half in range(2):
                            nc.tensor.matmul(
                                ps_s[0:8, half * 512:(half + 1) * 512],
                                e_all[:, 8 * hp: 8 * hp + 8],
                                u_all[:, hp * N + half * 512: hp * N + (half + 1) * 512],
                                start=(hp == 0), stop=(hp == H - 1),
                            )
                    pending_sums.append(h)
                while pending_sums:
                    hp = pending_sums.pop(0)
                    for half in range(2):
                        nc.tensor.matmul(
                            ps_s[0:8, half * 512:(half + 1) * 512],
                            e_all[:, 8 * hp: 8 * hp + 8],
                            u_all[:, hp * N + half * 512: hp * N + (half + 1) * 512],
                            start=(hp == 0), stop=(hp == H - 1),
                        )
                # drain any remaining projection chunks
                for _ in range(8):
                    drain_one_proj()
                return u_all, ps_s, next_qkv

            def norm_and_attn(u_all, ps_s, vp, bt):
                """reciprocal + r broadcast + attn@v + scaled evac -> outT."""
                r_f32 = smx_pool.tile([8, N], f32, name="r_f32", bufs=2)
                nc.vector.reciprocal_approx_fast(r_f32[:], ps_s[0:8, :])
                r_dr = dram_pool.tile([8, N], f32, name="r_dr", tag="r_dr", bufs=2)
                nc.gpsimd.dma_start(out=r_dr[:], in_=r_f32[:])
                rbc = []
                for pp in range(4):
                    t = smx_pool.tile([128, N], bf16, name=f"rbc{pp}", bufs=2)
                    for j in range(2):
                        nc.gpsimd.dma_start(
                            out=t[j * 64:(j + 1) * 64, :],
                            in_=r_dr[2 * pp + j: 2 * pp + j + 1, :].broadcast_to((64, N)),
                        )
                    rbc.append(t)

                outT = outt_pool.tile([128, CI * N], bf16, name="outT")
                for pp in range(4):
                    ps_o = psB.tile([128, N], f32, name="ps_o", tag="B", bufs=2)
                    for j in range(2):
                        h = 2 * pp + j
                        for half in range(2):
                            nc.tensor.matmul(
                                ps_o[j * 64:(j + 1) * 64, half * 512:(half + 1) * 512],
                                vp[:, h * 64:(h + 1) * 64],
                                u_all[:, h * N + half * 512: h * N + (half + 1) * 512],
                                start=True, stop=True,
                                tile_position=(0, j * 64),
                            )
                    nc.vector.tensor_tensor(
                        outT[:, pp * N:(pp + 1) * N], ps_o[:], rbc[pp][:], op=ALU.mult
                    )
                return outT

            def out_proj(outT, bt):
                """y = outT^T @ Wo + bo, two m-tiles per psum tile."""
                r0 = bt * N
                for mp in range(4):
                    ps_y = psA.tile([128, N], f32, name="ps_y", tag="A", bufs=2)
                    for half in range(2):
                        mt = 2 * mp + half
                        for pp in range(4):
                            nc.tensor.matmul(
                                ps_y[:, half * 512:(half + 1) * 512],
                                outT[:, pp * N + mt * 128: pp * N + (mt + 1) * 128],
                                w_sb["o"][:, pp * D:(pp + 1) * D],
                                start=(pp == 0),
                                stop=(pp == 3),
                            )
                    y_sb = y_pool.tile([128, N], f32, name="y_sb")
                    nc.vector.tensor_tensor(y_sb[:], ps_y[:], bo_bc2[:], op=ALU.add)
                    nc.gpsimd.dma_start(
                        out=y_flat[r0 + 2 * mp * 128: r0 + (2 * mp + 2) * 128, :]
                        .rearrange("(two p) d -> p two d", p=128),
                        in_=y_sb[:].rearrange("p (two d) -> p two d", two=2),
                    )

            # ---------------- main loop ----------------
            xTs = [stage_inputs(0, cast=False)]
            if n_bt > 1:
                xTs.append(stage_inputs(1, cast=False))
            # prologue: slice-0 projections up front
            qkv = None
            for r in proj_chunks(xTs[0], pool_reduce(xTs[0])):
                if r is not None:
                    qkv = r

            # out_proj for slice bt is emitted one iteration late (after
            # head_loop(bt+1)) so its y-adds queue behind the next slice's
            # u-mults on the DVE, instead of starving the PE's sums.
            pending = None
            for bt in range(n_bt):
                if bt + 2 < n_bt:
                    xTs.append(stage_inputs(bt + 2))
                gen = None
                if bt + 1 < n_bt:
                    gen = proj_chunks(xTs[bt + 1], pool_reduce(xTs[bt + 1]))
                u_all, ps_s, next_qkv = head_loop(qkv, gen)
                _, _, vp = qkv
                outT = norm_and_attn(u_all, ps_s, vp, bt)
                if pending is not None:
                    out_proj(*pending)
                pending = (outT, bt)
                qkv = next_qkv
            out_proj(*pending)

    return nc


_COMPILED = {}


def _get_compiled(n_bt=NBT):
    if n_bt not in _COMPILED:
        from concourse import bacc

        nc = bacc.Bacc("TRN2", target_bir_lowering=False, debug=False,
                       num_devices=NCORES)
        build_kernel(nc, n_bt)
        nc.compile()
        _COMPILED[n_bt] = nc
    return _COMPILED[n_bt]


def kernel(**inputs):
    """Full-input entry point: shards over batch across 8 cores."""
    os.environ.setdefault("JAX_PLATFORMS", "axon,cpu")
    os.environ.setdefault("NEURON_RT_RESET_CORES", "1")
    from concourse.bass_utils import run_bass_kernel_spmd

    nc = _get_compiled()

    x = np.ascontiguousarray(inputs["x"], dtype=np.float32)
    params = {
        k: np.ascontiguousarray(inputs[k], dtype=np.float32)
        for k in ("Wq", "bq", "Wk", "bk", "Wv", "bv", "Wo", "bo", "adp_pos")
    }
    in_maps = []
    for core in range(NCORES):
        m = {"x": x[core * BS:(core + 1) * BS]}
        m.update(params)
        in_maps.append(m)

    res = run_bass_kernel_spmd(nc, in_maps, core_ids=list(range(NCORES)))
    out = np.concatenate([res.results[i]["out"] for i in range(NCORES)], axis=0)
    return out


if __name__ == "__main__":
    import jax

    jax.config.update("jax_platforms", "cpu")
    import reference

    inputs = reference.setup_inputs()
    inputs = {k: np.asarray(v) for k, v in inputs.items()}
    expected = np.asarray(reference.reference(**inputs))
    actual = kernel(**inputs)
    err = np.linalg.norm(actual - expected) / np.linalg.norm(expected)
    print("Relative error:", err)


# revision 24
# speedup vs baseline: 1.2105x; 1.2105x over previous
"""Trainium2 Bass kernel for nn_AttentionLayer (pooled attention).

Reference computation (per batch b, step t):
    q = x @ Wq + bq                          # (N, D)
    k = mean-pool-8(x) @ Wk + bk             # (C, D)   [pool commutes with linear]
    v = mean-pool-8(x) @ Wv + bv             # (C, D)
    per head h (HD=64):
        score = qh @ khT / 8 + adp_pos       # (N, C)
        attn  = softmax(score, axis=-1)
        outh  = attn @ vh                    # (N, HD)
    y = concat(outh) @ Wo + bo               # (N, D)

Sharding: data-parallel over batch B=16 -> 2 per NeuronCore x 8 cores.
All matmuls in bf16 (f32 PSUM accumulation).

v2 design (vs v1): the softmax runs entirely in the TRANSPOSED
orientation, eliminating the per-slice attention DRAM round-trip +
xbar transpose that starved the PE in v1:
  - scores are computed directly as scT[c, n] (lhsT = kpT head slice,
    rhs = qT head slice, K=64).
  - u = exp(scT/8) * exp(adp)^T; softmax denominators come from
    matmuls with one-hot column tiles (e_all) accumulating all 8 heads
    into rows 0..7 of one PSUM tile; one reciprocal_approx_fast.
  - normalization is DEFERRED: o_unnorm^T = vp^T-slices @ u_T with the
    head pair packed into one PSUM tile via tile_position (0,0)/(0,64);
    the evacuation multiplies by r broadcast tiles (r spilled to DRAM
    and re-loaded with partition-replicating cast DMAs).
  - pooling is a strided DVE reduce over xT (no PE work).
  - bo is added during the y evacuation (DVE tensor_tensor add).
"""

import os

import numpy as np

B, T, N, D = 16, 12, 1024, 512
H, HD, C = 8, 64, 128
NCORES = 8
BS = B // NCORES          # batch per core
NBT = BS * T              # (b, t) slices per core
MT = N // 128             # m-tiles per (b, t) slice  = 8
CI = D // 128             # contraction chunks        = 4
POOL = N // C             # pooling factor            = 8


def build_kernel(nc, n_bt=NBT):
    """Emit the full per-core kernel graph into `nc` (a bacc.Bacc)."""
    import concourse.bass as bass
    import concourse.tile as tile
    from concourse import mybir

    f32 = mybir.dt.float32
    bf16 = mybir.dt.bfloat16
    AF = mybir.ActivationFunctionType
    ALU = mybir.AluOpType

    M = n_bt * N

    x_in = nc.dram_tensor("x", [BS, T, N, D], f32, kind="ExternalInput").ap()
    Wq_in = nc.dram_tensor("Wq", [D, D], f32, kind="ExternalInput").ap()
    bq_in = nc.dram_tensor("bq", [D], f32, kind="ExternalInput").ap()
    Wk_in = nc.dram_tensor("Wk", [D, D], f32, kind="ExternalInput").ap()
    bk_in = nc.dram_tensor("bk", [D], f32, kind="ExternalInput").ap()
    Wv_in = nc.dram_tensor("Wv", [D, D], f32, kind="ExternalInput").ap()
    bv_in = nc.dram_tensor("bv", [D], f32, kind="ExternalInput").ap()
    Wo_in = nc.dram_tensor("Wo", [D, D], f32, kind="ExternalInput").ap()
    bo_in = nc.dram_tensor("bo", [D], f32, kind="ExternalInput").ap()
    adp_in = nc.dram_tensor("adp_pos", [N, C], f32, kind="ExternalInput").ap()
    y_out = nc.dram_tensor("out", [BS, T, N, D], f32, kind="ExternalOutput").ap()

    x_flat = x_in.rearrange("b t n d -> (b t n) d")
    y_flat = y_out.rearrange("b t n d -> (b t n) d")

    with tile.TileContext(nc) as tc:
        with (
            tc.tile_pool(name="const", bufs=1) as const_pool,
            tc.tile_pool(name="dram", bufs=1, space="DRAM") as dram_pool,
            tc.tile_pool(name="xt", bufs=3) as xt_pool,
            tc.tile_pool(name="qt", bufs=2) as qt_pool,
            tc.tile_pool(name="pooled", bufs=2) as pooled_pool,
            tc.tile_pool(name="smx", bufs=2) as smx_pool,
            tc.tile_pool(name="outt", bufs=2) as outt_pool,
            tc.tile_pool(name="ysb", bufs=2) as y_pool,
            tc.tile_pool(name="psA", bufs=2, space="PSUM") as psA,
            tc.tile_pool(name="psB", bufs=2, space="PSUM") as psB,
        ):
            # bf16 copy of x in DRAM feeding the xbar transpose loads.
            # The first two slices' casts are issued before the weight
            # loads so the slice-0 transposes can start ASAP.
            x16 = dram_pool.tile([M, D], bf16, name="x16")

            # ---------------- constants / weights preload ----------------
            # W* layout: [128, ci*512 + dout] = W[ci*128 + p, dout]  (bf16)
            # Interleaved with the first two slices' x casts so the
            # critical prologue chain (x16 -> xT -> q-proj) starts ASAP.
            w_sb = {}

            def load_w(nm, w_ap):
                w_t = const_pool.tile([128, CI * D], bf16, name=f"W{nm}_sb")
                nc.gpsimd.dma_start(
                    out=w_t[:].rearrange("p (ci dout) -> p ci dout", ci=CI),
                    in_=w_ap.rearrange("(ci p) dout -> p ci dout", p=128),
                )
                w_sb[nm] = w_t

            for ci in range(CI):
                nc.gpsimd.dma_start(
                    out=x16[0:N, ci * 128:(ci + 1) * 128],
                    in_=x_flat[0:N, ci * 128:(ci + 1) * 128],
                )
            load_w("q", Wq_in)
            if n_bt > 1:
                for ci in range(CI):
                    nc.gpsimd.dma_start(
                        out=x16[N:2 * N, ci * 128:(ci + 1) * 128],
                        in_=x_flat[N:2 * N, ci * 128:(ci + 1) * 128],
                    )
            load_w("k", Wk_in)
            load_w("v", Wv_in)
            load_w("o", Wo_in)

            # per-partition bias tiles [128, dt] for the transposed q/k evacs
            bq_sb = const_pool.tile([128, CI], f32, name="bq_sb")
            nc.sync.dma_start(out=bq_sb[:], in_=bq_in.rearrange("(dt p) -> p dt", p=128))
            bk_sb = const_pool.tile([128, CI], f32, name="bk_sb")
            nc.sync.dma_start(out=bk_sb[:], in_=bk_in.rearrange("(dt p) -> p dt", p=128))

            # bv*8 row (vp psum holds 8*(xp_sum/8 @ Wv + bv); evac scales 1/8)
            bv_row = const_pool.tile([1, D], f32, name="bv_row")
            nc.sync.dma_start(out=bv_row[:], in_=bv_in.unsqueeze(0))
            bv8_row = const_pool.tile([1, D], bf16, name="bv8_row")
            nc.scalar.activation(bv8_row[:], bv_row[:], AF.Copy, scale=float(POOL))

            bo_row = const_pool.tile([1, D], f32, name="bo_row")
            nc.sync.dma_start(out=bo_row[:], in_=bo_in.unsqueeze(0))

            ones_k1 = const_pool.tile([1, 128], bf16, name="ones_k1")
            nc.vector.memset(ones_k1[:], 1.0)
            ones_f32 = const_pool.tile([1, 128], f32, name="ones_f32")
            nc.vector.memset(ones_f32[:], 1.0)

            # bo broadcast [128, 2*D] (two copies side by side, for the
            # two-mt y evacuation adds)
            bo_bc2 = const_pool.tile([128, 2 * D], f32, name="bo_bc2")
            ps_bo = psA.tile([128, 2 * D], f32, name="ps_bo", tag="A", bufs=2)
            for half in range(2):
                nc.tensor.matmul(
                    ps_bo[:, half * D:(half + 1) * D], ones_f32[:], bo_row[:],
                    start=True, stop=True,
                )
            nc.vector.tensor_copy(bo_bc2[:], ps_bo[:])

            # e_all: [128, 8*H]; slice h = cols [8h, 8h+8), ones in col h.
            # sums matmul lhsT -> head h's denominator lands in psum row h.
            e_all = const_pool.tile([128, 8 * H], bf16, name="e_all")
            nc.vector.memset(e_all[:], 0.0)
            for h in range(H):
                nc.vector.memset(e_all[:, 8 * h + h: 8 * h + h + 1], 1.0)

            # eadpT [c=128, n=N] = exp(adp_pos)^T  (bf16), built via natural
            # exp -> DRAM spill -> one xbar transpose.
            adp_f = const_pool.tile([128, MT * C], f32, name="adp_f")
            nc.sync.dma_start(
                out=adp_f[:].rearrange("p (nt c) -> p nt c", nt=MT),
                in_=adp_in.rearrange("(nt p) c -> p nt c", p=128),
            )
            eadp_nat = const_pool.tile([128, MT * C], bf16, name="eadp_nat")
            nc.scalar.activation(eadp_nat[:], adp_f[:], AF.Exp)
            eadp_dr = dram_pool.tile([N, C], bf16, name="eadp_dr")
            nc.sync.dma_start(
                out=eadp_dr[:].rearrange("(nt p) c -> p nt c", p=128),
                in_=eadp_nat[:].rearrange("p (nt c) -> p nt c", nt=MT),
            )
            eadpT = const_pool.tile([128, N], bf16, name="eadpT")
            nc.sync.dma_start(out=eadpT[:], in_=eadp_dr[:], transpose=True)

            # ---------------- per-slice emission helpers ----------------

            def stage_inputs(bt, cast=True):
                """x16 <- cast DMA (DRAM->DRAM, per column chunk so each
                xbar transpose can start as soon as its chunk lands);
                xT <- 4 xbar transposes."""
                r0 = bt * N
                xT = xt_pool.tile([128, CI * N], bf16, name="xT", tag="xT")
                for ci in range(CI):
                    if cast:
                        nc.gpsimd.dma_start(
                            out=x16[r0:r0 + N, ci * 128:(ci + 1) * 128],
                            in_=x_flat[r0:r0 + N, ci * 128:(ci + 1) * 128],
                        )
                    nc.sync.dma_start(
                        out=xT[:, ci * N:(ci + 1) * N],
                        in_=x16[r0:r0 + N, ci * 128:(ci + 1) * 128],
                        transpose=True,
                    )
                return xT

            def proj_chunks(xT):
                """Generator of PE work for one slice's projections; yields
                between chunks so the caller interleaves it into the softmax
                head loop. Final yield returns (qT, kpT, vp)."""
                qT = qt_pool.tile([128, CI * N], bf16, name="qT")
                for dt in range(CI):
                    ps_q = psA.tile([128, N], f32, name="ps_q", tag="A", bufs=2)
                    # ci outer / mc inner: consecutive matmuls share the
                    # stationary tile, letting LDWEIGHTS be skipped/hidden
                    for ci in range(CI):
                        for mc in range(2):
                            nc.tensor.matmul(
                                ps_q[:, mc * 512:(mc + 1) * 512],
                                w_sb["q"][:, ci * D + dt * 128: ci * D + dt * 128 + 128],
                                xT[:, ci * N + mc * 512: ci * N + mc * 512 + 512],
                                start=(ci == 0),
                                stop=(ci == CI - 1),
                            )
                    nc.scalar.activation(
                        qT[:, dt * N:(dt + 1) * N], ps_q[:],
                        AF.Identity, bias=bq_sb[:, dt:dt + 1],
                    )
                    yield None

                xpT = pooled_pool.tile([128, CI * C], bf16, name="xpT")
                with nc.allow_low_precision("pool sums, bf16 ok"):
                    for ci in range(CI):
                        nc.vector.reduce_sum(
                            xpT[:, ci * C:(ci + 1) * C],
                            xT[:, ci * N:(ci + 1) * N]
                            .rearrange("p (j i) -> p j i", i=POOL),
                            axis=mybir.AxisListType.X,
                        )

                # kp (transposed) + vp (natural) share one psum tile
                ps_kv = psB.tile([128, N], f32, name="ps_kv", tag="B", bufs=2)
                kpT = pooled_pool.tile([128, CI * C], bf16, name="kpT")
                vp = pooled_pool.tile([128, D], bf16, name="vp")
                for dt in range(CI):
                    for ci in range(CI):
                        nc.tensor.matmul(
                            ps_kv[:, dt * 128:(dt + 1) * 128],
                            w_sb["k"][:, ci * D + dt * 128: ci * D + dt * 128 + 128],
                            xpT[:, ci * C:(ci + 1) * C],
                            start=(ci == 0),
                            stop=(ci == CI - 1),
                        )
                yield None
                for ci in range(CI):
                    nc.tensor.matmul(
                        ps_kv[:, 512:],
                        xpT[:, ci * C:(ci + 1) * C],
                        w_sb["v"][:, ci * D:(ci + 1) * D],
                        start=(ci == 0),
                        stop=False,
                    )
                nc.tensor.matmul(
                    ps_kv[:, 512:], ones_k1[:], bv8_row[:], start=False, stop=True
                )
                for dt in range(CI):
                    nc.scalar.activation(
                        kpT[:, dt * 128:(dt + 1) * 128],
                        ps_kv[:, dt * 128:(dt + 1) * 128],
                        AF.Identity, scale=1.0 / POOL, bias=bk_sb[:, dt:dt + 1],
                    )
                nc.scalar.activation(vp[:], ps_kv[:, 512:], AF.Copy, scale=1.0 / POOL)
                yield (qT, kpT, vp)

            def head_loop(qkv, proj_gen):
                """Scores/exp/u/sums for all 8 heads of one slice, with the
                NEXT slice's q-projection chunks interleaved into the PE
                gaps. Returns (u_all, ps_s, next_qT)."""
                qT, kpT, vp = qkv
                u_all = smx_pool.tile([128, H * N], bf16, name="u_all", bufs=1)
                ps_s = psB.tile([128, N], f32, name="ps_s", tag="B", bufs=2)
                next_qkv = None
                pending_sums = []

                def drain_one_proj():
                    nonlocal next_qkv
                    if proj_gen is not None:
                        try:
                            r = next(proj_gen)
                            if r is not None:
                                next_qkv = r
                        except StopIteration:
                            pass

                for h in range(H):
                    ph = (h % 2) * 64
                    dt = h // 2
                    ps_sc = psA.tile([128, N], f32, name="ps_sc", tag="A", bufs=2)
                    for half in range(2):
                        nc.tensor.matmul(
                            ps_sc[:, half * 512:(half + 1) * 512],
                            kpT[ph:ph + 64, dt * 128:(dt + 1) * 128],
                            qT[ph:ph + 64, dt * N + half * 512: dt * N + (half + 1) * 512],
                            start=True, stop=True,
                        )
                    exp_sb = smx_pool.tile([128, N], bf16, name="exp_sb", bufs=2)
                    nc.scalar.activation(exp_sb[:], ps_sc[:], AF.Exp, scale=1.0 / 8.0)
                    nc.vector.tensor_tensor(
                        u_all[:, h * N:(h + 1) * N], exp_sb[:], eadpT[:], op=ALU.mult
                    )
                    # interleave projection work into the exp/u latency gap
                    drain_one_proj()
                    if pending_sums:
                        hp = pending_sums.pop(0)
                        for half in range(2):
                            nc.tensor.matmul(
                                ps_s[0:8, half * 512:(half + 1) * 512],
                                e_all[:, 8 * hp: 8 * hp + 8],
                                u_all[:, hp * N + half * 512: hp * N + (half + 1) * 512],
                                start=(hp == 0), stop=(hp == H - 1),
                            )
                    pending_sums.append(h)
                while pending_sums:
                    hp = pending_sums.pop(0)
                    for half in range(2):
                        nc.tensor.matmul(
                            ps_s[0:8, half * 512:(half + 1) * 512],
                            e_all[:, 8 * hp: 8 * hp + 8],
                            u_all[:, hp * N + half * 512: hp * N + (half + 1) * 512],
                            start=(hp == 0), stop=(hp == H - 1),
                        )
                # drain any remaining projection chunks
                for _ in range(8):
                    drain_one_proj()
                return u_all, ps_s, next_qkv

            def norm_and_attn(u_all, ps_s, vp, bt):
                """reciprocal + r broadcast + attn@v + scaled evac -> outT."""
                r_f32 = smx_pool.tile([8, N], f32, name="r_f32", bufs=2)
                nc.vector.reciprocal_approx_fast(r_f32[:], ps_s[0:8, :])
                r_dr = dram_pool.tile([8, N], f32, name="r_dr", tag="r_dr", bufs=2)
                nc.gpsimd.dma_start(out=r_dr[:], in_=r_f32[:])
                rbc = []
                for pp in range(4):
                    t = smx_pool.tile([128, N], bf16, name=f"rbc{pp}", bufs=2)
                    for j in range(2):
                        nc.gpsimd.dma_start(
                            out=t[j * 64:(j + 1) * 64, :],
                            in_=r_dr[2 * pp + j: 2 * pp + j + 1, :].broadcast_to((64, N)),
                        )
                    rbc.append(t)

                outT = outt_pool.tile([128, CI * N], bf16, name="outT")
                for pp in range(4):
                    ps_o = psB.tile([128, N], f32, name="ps_o", tag="B", bufs=2)
                    for j in range(2):
                        h = 2 * pp + j
                        for half in range(2):
                            nc.tensor.matmul(
                                ps_o[j * 64:(j + 1) * 64, half * 512:(half + 1) * 512],
                                vp[:, h * 64:(h + 1) * 64],
                                u_all[:, h * N + half * 512: h * N + (half + 1) * 512],
                                start=True, stop=True,
                                tile_position=(0, j * 64),
                            )
                    nc.vector.tensor_tensor(
                        outT[:, pp * N:(pp + 1) * N], ps_o[:], rbc[pp][:], op=ALU.mult
                    )
                return outT

            def out_proj(outT, bt):
                """y = outT^T @ Wo + bo, two m-tiles per psum tile."""
                r0 = bt * N
                for mp in range(4):
                    ps_y = psA.tile([128, N], f32, name="ps_y", tag="A", bufs=2)
                    for half in range(2):
                        mt = 2 * mp + half
                        for pp in range(4):
                            nc.tensor.matmul(
                                ps_y[:, half * 512:(half + 1) * 512],
                                outT[:, pp * N + mt * 128: pp * N + (mt + 1) * 128],
                                w_sb["o"][:, pp * D:(pp + 1) * D],
                                start=(pp == 0),
                                stop=(pp == 3),
                            )
                    y_sb = y_pool.tile([128, N], f32, name="y_sb")
                    nc.vector.tensor_tensor(y_sb[:], ps_y[:], bo_bc2[:], op=ALU.add)
                    nc.gpsimd.dma_start(
                        out=y_flat[r0 + 2 * mp * 128: r0 + (2 * mp + 2) * 128, :]
                        .rearrange("(two p) d -> p two d", p=128),
                        in_=y_sb[:].rearrange("p (two d) -> p two d", two=2),
                    )

            # ---------------- main loop ----------------
            xTs = [stage_inputs(0, cast=False)]
            if n_bt > 1:
                xTs.append(stage_inputs(1, cast=False))
            # prologue: slice-0 projections up front
            qkv = None
            for r in proj_chunks(xTs[0]):
                if r is not None:
                    qkv = r

            # out_proj for slice bt is emitted one iteration late (after
            # head_loop(bt+1)) so its y-adds queue behind the next slice's
            # u-mults on the DVE, instead of starving the PE's sums.
            pending = None
            for bt in range(n_bt):
                if bt + 2 < n_bt:
                    xTs.append(stage_inputs(bt + 2))
                gen = proj_chunks(xTs[bt + 1]) if bt + 1 < n_bt else None
                u_all, ps_s, next_qkv = head_loop(qkv, gen)
                _, _, vp = qkv
                outT = norm_and_attn(u_all, ps_s, vp, bt)
                if pending is not None:
                    out_proj(*pending)
                pending = (outT, bt)
                qkv = next_qkv
            out_proj(*pending)

    return nc


_COMPILED = {}


def _get_compiled(n_bt=NBT):
    if n_bt not in _COMPILED:
        from concourse import bacc

        nc = bacc.Bacc("TRN2", target_bir_lowering=False, debug=False,
                       num_devices=NCORES)
        build_kernel(nc, n_bt)
        nc.compile()
        _COMPILED[n_bt] = nc
    return _COMPILED[n_bt]


def kernel(**inputs):
    """Full-input entry point: shards over batch across 8 cores."""
    os.environ.setdefault("JAX_PLATFORMS", "axon,cpu")
    os.environ.setdefault("NEURON_RT_RESET_CORES", "1")
    from concourse.bass_utils import run_bass_kernel_spmd

    nc = _get_compiled()

    x = np.ascontiguousarray(inputs["x"], dtype=np.float32)
    params = {
        k: np.ascontiguousarray(inputs[k], dtype=np.float32)
        for k in ("Wq", "bq", "Wk", "bk", "Wv", "bv", "Wo", "bo", "adp_pos")
    }
    in_maps = []
    for core in range(NCORES):
        m = {"x": x[core * BS:(core + 1) * BS]}
        m.update(params)
        in_maps.append(m)

    res = run_bass_kernel_spmd(nc, in_maps, core_ids=list(range(NCORES)))
    out = np.concatenate([res.results[i]["out"] for i in range(NCORES)], axis=0)
    return out


if __name__ == "__main__":
    import jax

    jax.config.update("jax_platforms", "cpu")
    import reference

    inputs = reference.setup_inputs()
    inputs = {k: np.asarray(v) for k, v in inputs.items()}
    expected = np.asarray(reference.reference(**inputs))
    actual = kernel(**inputs)
    err = np.linalg.norm(actual - expected) / np.linalg.norm(expected)
    print("Relative error:", err)


# revision 25
# speedup vs baseline: 1.2567x; 1.0381x over previous
"""Trainium2 Bass kernel for nn_AttentionLayer (pooled attention).

Reference computation (per batch b, step t):
    q = x @ Wq + bq                          # (N, D)
    k = mean-pool-8(x) @ Wk + bk             # (C, D)   [pool commutes with linear]
    v = mean-pool-8(x) @ Wv + bv             # (C, D)
    per head h (HD=64):
        score = qh @ khT / 8 + adp_pos       # (N, C)
        attn  = softmax(score, axis=-1)
        outh  = attn @ vh                    # (N, HD)
    y = concat(outh) @ Wo + bo               # (N, D)

Sharding: data-parallel over batch B=16 -> 2 per NeuronCore x 8 cores.
All matmuls in bf16 (f32 PSUM accumulation).

v2 design (vs v1): the softmax runs entirely in the TRANSPOSED
orientation, eliminating the per-slice attention DRAM round-trip +
xbar transpose that starved the PE in v1:
  - scores are computed directly as scT[c, n] (lhsT = kpT head slice,
    rhs = qT head slice, K=64).
  - u = exp(scT/8) * exp(adp)^T; softmax denominators come from
    matmuls with one-hot column tiles (e_all) accumulating all 8 heads
    into rows 0..7 of one PSUM tile; one reciprocal_approx_fast.
  - normalization is DEFERRED: o_unnorm^T = vp^T-slices @ u_T with the
    head pair packed into one PSUM tile via tile_position (0,0)/(0,64);
    the evacuation multiplies by r broadcast tiles (r spilled to DRAM
    and re-loaded with partition-replicating cast DMAs).
  - pooling is a strided DVE reduce over xT (no PE work).
  - bo is added during the y evacuation (DVE tensor_tensor add).
"""

import os

import numpy as np

B, T, N, D = 16, 12, 1024, 512
H, HD, C = 8, 64, 128
NCORES = 8
BS = B // NCORES          # batch per core
NBT = BS * T              # (b, t) slices per core
MT = N // 128             # m-tiles per (b, t) slice  = 8
CI = D // 128             # contraction chunks        = 4
POOL = N // C             # pooling factor            = 8


def build_kernel(nc, n_bt=NBT):
    """Emit the full per-core kernel graph into `nc` (a bacc.Bacc)."""
    import concourse.bass as bass
    import concourse.tile as tile
    from concourse import mybir

    f32 = mybir.dt.float32
    bf16 = mybir.dt.bfloat16
    AF = mybir.ActivationFunctionType
    ALU = mybir.AluOpType

    M = n_bt * N

    x_in = nc.dram_tensor("x", [BS, T, N, D], f32, kind="ExternalInput").ap()
    Wq_in = nc.dram_tensor("Wq", [D, D], f32, kind="ExternalInput").ap()
    bq_in = nc.dram_tensor("bq", [D], f32, kind="ExternalInput").ap()
    Wk_in = nc.dram_tensor("Wk", [D, D], f32, kind="ExternalInput").ap()
    bk_in = nc.dram_tensor("bk", [D], f32, kind="ExternalInput").ap()
    Wv_in = nc.dram_tensor("Wv", [D, D], f32, kind="ExternalInput").ap()
    bv_in = nc.dram_tensor("bv", [D], f32, kind="ExternalInput").ap()
    Wo_in = nc.dram_tensor("Wo", [D, D], f32, kind="ExternalInput").ap()
    bo_in = nc.dram_tensor("bo", [D], f32, kind="ExternalInput").ap()
    adp_in = nc.dram_tensor("adp_pos", [N, C], f32, kind="ExternalInput").ap()
    y_out = nc.dram_tensor("out", [BS, T, N, D], f32, kind="ExternalOutput").ap()

    x_flat = x_in.rearrange("b t n d -> (b t n) d")
    y_flat = y_out.rearrange("b t n d -> (b t n) d")

    with tile.TileContext(nc) as tc:
        with (
            tc.tile_pool(name="const", bufs=1) as const_pool,
            tc.tile_pool(name="dram", bufs=1, space="DRAM") as dram_pool,
            tc.tile_pool(name="xt", bufs=3) as xt_pool,
            tc.tile_pool(name="qt", bufs=2) as qt_pool,
            tc.tile_pool(name="pooled", bufs=2) as pooled_pool,
            tc.tile_pool(name="smx", bufs=2) as smx_pool,
            tc.tile_pool(name="outt", bufs=2) as outt_pool,
            tc.tile_pool(name="ysb", bufs=2) as y_pool,
            tc.tile_pool(name="psA", bufs=2, space="PSUM") as psA,
            tc.tile_pool(name="psB", bufs=2, space="PSUM") as psB,
        ):
            # bf16 copy of x in DRAM feeding the xbar transpose loads.
            # The first two slices' casts are issued before the weight
            # loads so the slice-0 transposes can start ASAP.
            x16 = dram_pool.tile([M, D], bf16, name="x16")

            # ---------------- constants / weights preload ----------------
            # W* layout: [128, ci*512 + dout] = W[ci*128 + p, dout]  (bf16)
            # Interleaved with the first two slices' x casts so the
            # critical prologue chain (x16 -> xT -> q-proj) starts ASAP.
            w_sb = {}

            def load_w(nm, w_ap):
                w_t = const_pool.tile([128, CI * D], bf16, name=f"W{nm}_sb")
                nc.gpsimd.dma_start(
                    out=w_t[:].rearrange("p (ci dout) -> p ci dout", ci=CI),
                    in_=w_ap.rearrange("(ci p) dout -> p ci dout", p=128),
                )
                w_sb[nm] = w_t

            for ci in range(CI):
                nc.gpsimd.dma_start(
                    out=x16[0:N, ci * 128:(ci + 1) * 128],
                    in_=x_flat[0:N, ci * 128:(ci + 1) * 128],
                )
            load_w("q", Wq_in)
            if n_bt > 1:
                for ci in range(CI):
                    nc.gpsimd.dma_start(
                        out=x16[N:2 * N, ci * 128:(ci + 1) * 128],
                        in_=x_flat[N:2 * N, ci * 128:(ci + 1) * 128],
                    )
            load_w("k", Wk_in)
            load_w("v", Wv_in)
            load_w("o", Wo_in)

            # per-partition bias tiles [128, dt] for the transposed q/k evacs
            bq_sb = const_pool.tile([128, CI], f32, name="bq_sb")
            nc.sync.dma_start(out=bq_sb[:], in_=bq_in.rearrange("(dt p) -> p dt", p=128))
            bk_sb = const_pool.tile([128, CI], f32, name="bk_sb")
            nc.sync.dma_start(out=bk_sb[:], in_=bk_in.rearrange("(dt p) -> p dt", p=128))

            # bv*8 row (vp psum holds 8*(xp_sum/8 @ Wv + bv); evac scales 1/8)
            bv_row = const_pool.tile([1, D], f32, name="bv_row")
            nc.sync.dma_start(out=bv_row[:], in_=bv_in.unsqueeze(0))
            bv8_row = const_pool.tile([1, D], bf16, name="bv8_row")
            nc.scalar.activation(bv8_row[:], bv_row[:], AF.Copy, scale=float(POOL))

            bo_row = const_pool.tile([1, D], f32, name="bo_row")
            nc.sync.dma_start(out=bo_row[:], in_=bo_in.unsqueeze(0))

            ones_k1 = const_pool.tile([1, 128], bf16, name="ones_k1")
            nc.vector.memset(ones_k1[:], 1.0)
            ones_f32 = const_pool.tile([1, 128], f32, name="ones_f32")
            nc.vector.memset(ones_f32[:], 1.0)

            # bo broadcast [128, 2*D] (two copies side by side, for the
            # two-mt y evacuation adds)
            bo_bc2 = const_pool.tile([128, 2 * D], f32, name="bo_bc2")
            ps_bo = psA.tile([128, 2 * D], f32, name="ps_bo", tag="A", bufs=2)
            for half in range(2):
                nc.tensor.matmul(
                    ps_bo[:, half * D:(half + 1) * D], ones_f32[:], bo_row[:],
                    start=True, stop=True,
                )
            nc.vector.tensor_copy(bo_bc2[:], ps_bo[:])

            # e_all: [128, 8*H]; slice h = cols [8h, 8h+8), ones in col h.
            # sums matmul lhsT -> head h's denominator lands in psum row h.
            e_all = const_pool.tile([128, 8 * H], bf16, name="e_all")
            nc.vector.memset(e_all[:], 0.0)
            for h in range(H):
                nc.vector.memset(e_all[:, 8 * h + h: 8 * h + h + 1], 1.0)

            # eadpT [c=128, n=N] = exp(adp_pos)^T  (bf16), built via natural
            # exp -> DRAM spill -> one xbar transpose.
            adp_f = const_pool.tile([128, MT * C], f32, name="adp_f")
            nc.sync.dma_start(
                out=adp_f[:].rearrange("p (nt c) -> p nt c", nt=MT),
                in_=adp_in.rearrange("(nt p) c -> p nt c", p=128),
            )
            eadp_nat = const_pool.tile([128, MT * C], bf16, name="eadp_nat")
            nc.scalar.activation(eadp_nat[:], adp_f[:], AF.Exp)
            eadp_dr = dram_pool.tile([N, C], bf16, name="eadp_dr")
            nc.sync.dma_start(
                out=eadp_dr[:].rearrange("(nt p) c -> p nt c", p=128),
                in_=eadp_nat[:].rearrange("p (nt c) -> p nt c", nt=MT),
            )
            eadpT = const_pool.tile([128, N], bf16, name="eadpT")
            nc.sync.dma_start(out=eadpT[:], in_=eadp_dr[:], transpose=True)

            # ---------------- per-slice emission helpers ----------------

            def stage_inputs(bt, cast=True):
                """x16 <- cast DMA (DRAM->DRAM, per column chunk so each
                xbar transpose can start as soon as its chunk lands);
                xT <- 4 xbar transposes."""
                r0 = bt * N
                xT = xt_pool.tile([128, CI * N], bf16, name="xT", tag="xT")
                for ci in range(CI):
                    if cast:
                        nc.gpsimd.dma_start(
                            out=x16[r0:r0 + N, ci * 128:(ci + 1) * 128],
                            in_=x_flat[r0:r0 + N, ci * 128:(ci + 1) * 128],
                        )
                    nc.sync.dma_start(
                        out=xT[:, ci * N:(ci + 1) * N],
                        in_=x16[r0:r0 + N, ci * 128:(ci + 1) * 128],
                        transpose=True,
                    )
                return xT

            def proj_chunks(xT, kv_ps_fn):
                """Generator of PE work for one slice's projections; yields
                between chunks so the caller interleaves it into the softmax
                head loop. Final yield returns (qT, kpT, vp)."""
                qT = qt_pool.tile([128, CI * N], bf16, name="qT")
                for dt in range(CI):
                    ps_q = psA.tile([128, N], f32, name="ps_q", tag="A", bufs=2)
                    # ci outer / mc inner: consecutive matmuls share the
                    # stationary tile, letting LDWEIGHTS be skipped/hidden
                    for ci in range(CI):
                        for mc in range(2):
                            nc.tensor.matmul(
                                ps_q[:, mc * 512:(mc + 1) * 512],
                                w_sb["q"][:, ci * D + dt * 128: ci * D + dt * 128 + 128],
                                xT[:, ci * N + mc * 512: ci * N + mc * 512 + 512],
                                start=(ci == 0),
                                stop=(ci == CI - 1),
                            )
                    nc.scalar.activation(
                        qT[:, dt * N:(dt + 1) * N], ps_q[:],
                        AF.Identity, bias=bq_sb[:, dt:dt + 1],
                    )
                    yield None

                xpT = pooled_pool.tile([128, CI * C], bf16, name="xpT")
                with nc.allow_low_precision("pool sums, bf16 ok"):
                    for ci in range(CI):
                        nc.vector.reduce_sum(
                            xpT[:, ci * C:(ci + 1) * C],
                            xT[:, ci * N:(ci + 1) * N]
                            .rearrange("p (j i) -> p j i", i=POOL),
                            axis=mybir.AxisListType.X,
                        )

                # kp (transposed) + vp (natural) share one psum tile,
                # allocated by the caller BEFORE ps_s so the psB ring pairs
                # the attn@v tiles with this early-freed tile instead of
                # ps_s (whose release needs the reciprocal).
                ps_kv = kv_ps_fn()
                kpT = pooled_pool.tile([128, CI * C], bf16, name="kpT")
                vp = pooled_pool.tile([128, D], bf16, name="vp")
                for dt in range(CI):
                    for ci in range(CI):
                        nc.tensor.matmul(
                            ps_kv[:, dt * 128:(dt + 1) * 128],
                            w_sb["k"][:, ci * D + dt * 128: ci * D + dt * 128 + 128],
                            xpT[:, ci * C:(ci + 1) * C],
                            start=(ci == 0),
                            stop=(ci == CI - 1),
                        )
                yield None
                for ci in range(CI):
                    nc.tensor.matmul(
                        ps_kv[:, 512:],
                        xpT[:, ci * C:(ci + 1) * C],
                        w_sb["v"][:, ci * D:(ci + 1) * D],
                        start=(ci == 0),
                        stop=False,
                    )
                nc.tensor.matmul(
                    ps_kv[:, 512:], ones_k1[:], bv8_row[:], start=False, stop=True
                )
                for dt in range(CI):
                    nc.scalar.activation(
                        kpT[:, dt * 128:(dt + 1) * 128],
                        ps_kv[:, dt * 128:(dt + 1) * 128],
                        AF.Identity, scale=1.0 / POOL, bias=bk_sb[:, dt:dt + 1],
                    )
                nc.scalar.activation(vp[:], ps_kv[:, 512:], AF.Copy, scale=1.0 / POOL)
                yield (qT, kpT, vp)

            def head_loop(qkv, proj_gen):
                """Scores/exp/u/sums for all 8 heads of one slice, with the
                NEXT slice's q-projection chunks interleaved into the PE
                gaps. Returns (u_all, ps_s, next_qT)."""
                qT, kpT, vp = qkv
                u_all = smx_pool.tile([128, H * N], bf16, name="u_all", bufs=1)
                if proj_gen is not None:
                    kv_holder["t"] = psB.tile([128, N], f32, name="ps_kv",
                                              tag="B", bufs=2)
                ps_s = psB.tile([128, N], f32, name="ps_s", tag="B", bufs=2)
                next_qkv = None
                pending_sums = []

                def drain_one_proj():
                    nonlocal next_qkv
                    if proj_gen is not None:
                        try:
                            r = next(proj_gen)
                            if r is not None:
                                next_qkv = r
                        except StopIteration:
                            pass

                for h in range(H):
                    ph = (h % 2) * 64
                    dt = h // 2
                    ps_sc = psA.tile([128, N], f32, name="ps_sc", tag="A", bufs=2)
                    for half in range(2):
                        nc.tensor.matmul(
                            ps_sc[:, half * 512:(half + 1) * 512],
                            kpT[ph:ph + 64, dt * 128:(dt + 1) * 128],
                            qT[ph:ph + 64, dt * N + half * 512: dt * N + (half + 1) * 512],
                            start=True, stop=True,
                        )
                    exp_sb = smx_pool.tile([128, N], bf16, name="exp_sb", bufs=2)
                    nc.scalar.activation(exp_sb[:], ps_sc[:], AF.Exp, scale=1.0 / 8.0)
                    nc.vector.tensor_tensor(
                        u_all[:, h * N:(h + 1) * N], exp_sb[:], eadpT[:], op=ALU.mult
                    )
                    # interleave projection work into the exp/u latency gap
                    drain_one_proj()
                    if pending_sums:
                        hp = pending_sums.pop(0)
                        for half in range(2):
                            nc.tensor.matmul(
                                ps_s[0:8, half * 512:(half + 1) * 512],
                                e_all[:, 8 * hp: 8 * hp + 8],
                                u_all[:, hp * N + half * 512: hp * N + (half + 1) * 512],
                                start=(hp == 0), stop=(hp == H - 1),
                            )
                    pending_sums.append(h)
                while pending_sums:
                    hp = pending_sums.pop(0)
                    for half in range(2):
                        nc.tensor.matmul(
                            ps_s[0:8, half * 512:(half + 1) * 512],
                            e_all[:, 8 * hp: 8 * hp + 8],
                            u_all[:, hp * N + half * 512: hp * N + (half + 1) * 512],
                            start=(hp == 0), stop=(hp == H - 1),
                        )
                # drain any remaining projection chunks
                for _ in range(8):
                    drain_one_proj()
                return u_all, ps_s, next_qkv

            def norm_and_attn(u_all, ps_s, vp, bt):
                """reciprocal + r broadcast + attn@v + scaled evac -> outT."""
                r_f32 = smx_pool.tile([8, N], f32, name="r_f32", bufs=2)
                nc.vector.reciprocal_approx_fast(r_f32[:], ps_s[0:8, :])
                r_dr = dram_pool.tile([8, N], f32, name="r_dr", tag="r_dr", bufs=2)
                nc.gpsimd.dma_start(out=r_dr[:], in_=r_f32[:])
                rbc = []
                for pp in range(4):
                    t = smx_pool.tile([128, N], bf16, name=f"rbc{pp}", bufs=2)
                    for j in range(2):
                        nc.gpsimd.dma_start(
                            out=t[j * 64:(j + 1) * 64, :],
                            in_=r_dr[2 * pp + j: 2 * pp + j + 1, :].broadcast_to((64, N)),
                        )
                    rbc.append(t)

                outT = outt_pool.tile([128, CI * N], bf16, name="outT")
                for pp in range(4):
                    ps_o = psB.tile([128, N], f32, name="ps_o", tag="B", bufs=2)
                    for j in range(2):
                        h = 2 * pp + j
                        for half in range(2):
                            nc.tensor.matmul(
                                ps_o[j * 64:(j + 1) * 64, half * 512:(half + 1) * 512],
                                vp[:, h * 64:(h + 1) * 64],
                                u_all[:, h * N + half * 512: h * N + (half + 1) * 512],
                                start=True, stop=True,
                                tile_position=(0, j * 64),
                            )
                    nc.vector.tensor_tensor(
                        outT[:, pp * N:(pp + 1) * N], ps_o[:], rbc[pp][:], op=ALU.mult
                    )
                return outT

            def out_proj(outT, bt):
                """y = outT^T @ Wo + bo, two m-tiles per psum tile."""
                r0 = bt * N
                for mp in range(4):
                    ps_y = psA.tile([128, N], f32, name="ps_y", tag="A", bufs=2)
                    for half in range(2):
                        mt = 2 * mp + half
                        for pp in range(4):
                            nc.tensor.matmul(
                                ps_y[:, half * 512:(half + 1) * 512],
                                outT[:, pp * N + mt * 128: pp * N + (mt + 1) * 128],
                                w_sb["o"][:, pp * D:(pp + 1) * D],
                                start=(pp == 0),
                                stop=(pp == 3),
                            )
                    y_sb = y_pool.tile([128, N], f32, name="y_sb")
                    nc.vector.tensor_tensor(y_sb[:], ps_y[:], bo_bc2[:], op=ALU.add)
                    nc.gpsimd.dma_start(
                        out=y_flat[r0 + 2 * mp * 128: r0 + (2 * mp + 2) * 128, :]
                        .rearrange("(two p) d -> p two d", p=128),
                        in_=y_sb[:].rearrange("p (two d) -> p two d", two=2),
                    )

            # ---------------- main loop ----------------
            kv_holder = {}
            xTs = [stage_inputs(0, cast=False)]
            if n_bt > 1:
                xTs.append(stage_inputs(1, cast=False))
            # prologue: slice-0 projections up front
            kv_holder["t"] = psB.tile([128, N], f32, name="ps_kv",
                                      tag="B", bufs=2)
            qkv = None
            for r in proj_chunks(xTs[0], lambda: kv_holder.pop("t")):
                if r is not None:
                    qkv = r

            # out_proj for slice bt is emitted one iteration late (after
            # head_loop(bt+1)) so its y-adds queue behind the next slice's
            # u-mults on the DVE, instead of starving the PE's sums.
            pending = None
            for bt in range(n_bt):
                if bt + 2 < n_bt:
                    xTs.append(stage_inputs(bt + 2))
                gen = None
                if bt + 1 < n_bt:
                    gen = proj_chunks(xTs[bt + 1], lambda: kv_holder.pop("t"))
                u_all, ps_s, next_qkv = head_loop(qkv, gen)
                _, _, vp = qkv
                outT = norm_and_attn(u_all, ps_s, vp, bt)
                if pending is not None:
                    out_proj(*pending)
                pending = (outT, bt)
                qkv = next_qkv
            out_proj(*pending)

    return nc


_COMPILED = {}


def _get_compiled(n_bt=NBT):
    if n_bt not in _COMPILED:
        from concourse import bacc

        nc = bacc.Bacc("TRN2", target_bir_lowering=False, debug=False,
                       num_devices=NCORES)
        build_kernel(nc, n_bt)
        nc.compile()
        _COMPILED[n_bt] = nc
    return _COMPILED[n_bt]


def kernel(**inputs):
    """Full-input entry point: shards over batch across 8 cores."""
    os.environ.setdefault("JAX_PLATFORMS", "axon,cpu")
    os.environ.setdefault("NEURON_RT_RESET_CORES", "1")
    from concourse.bass_utils import run_bass_kernel_spmd

    nc = _get_compiled()

    x = np.ascontiguousarray(inputs["x"], dtype=np.float32)
    params = {
        k: np.ascontiguousarray(inputs[k], dtype=np.float32)
        for k in ("Wq", "bq", "Wk", "bk", "Wv", "bv", "Wo", "bo", "adp_pos")
    }
    in_maps = []
    for core in range(NCORES):
        m = {"x": x[core * BS:(core + 1) * BS]}
        m.update(params)
        in_maps.append(m)

    res = run_bass_kernel_spmd(nc, in_maps, core_ids=list(range(NCORES)))
    out = np.concatenate([res.results[i]["out"] for i in range(NCORES)], axis=0)
    return out


if __name__ == "__main__":
    import jax

    jax.config.update("jax_platforms", "cpu")
    import reference

    inputs = reference.setup_inputs()
    inputs = {k: np.asarray(v) for k, v in inputs.items()}
    expected = np.asarray(reference.reference(**inputs))
    actual = kernel(**inputs)
    err = np.linalg.norm(actual - expected) / np.linalg.norm(expected)
    print("Relative error:", err)


# revision 26
# speedup vs baseline: 1.2711x; 1.0115x over previous
"""Trainium2 Bass kernel for nn_AttentionLayer (pooled attention).

Reference computation (per batch b, step t):
    q = x @ Wq + bq                          # (N, D)
    k = mean-pool-8(x) @ Wk + bk             # (C, D)   [pool commutes with linear]
    v = mean-pool-8(x) @ Wv + bv             # (C, D)
    per head h (HD=64):
        score = qh @ khT / 8 + adp_pos       # (N, C)
        attn  = softmax(score, axis=-1)
        outh  = attn @ vh                    # (N, HD)
    y = concat(outh) @ Wo + bo               # (N, D)

Sharding: data-parallel over batch B=16 -> 2 per NeuronCore x 8 cores.
All matmuls in bf16 (f32 PSUM accumulation).

v2 design (vs v1): the softmax runs entirely in the TRANSPOSED
orientation, eliminating the per-slice attention DRAM round-trip +
xbar transpose that starved the PE in v1:
  - scores are computed directly as scT[c, n] (lhsT = kpT head slice,
    rhs = qT head slice, K=64).
  - u = exp(scT/8) * exp(adp)^T; softmax denominators come from
    matmuls with one-hot column tiles (e_all) accumulating all 8 heads
    into rows 0..7 of one PSUM tile; one reciprocal_approx_fast.
  - normalization is DEFERRED: o_unnorm^T = vp^T-slices @ u_T with the
    head pair packed into one PSUM tile via tile_position (0,0)/(0,64);
    the evacuation multiplies by r broadcast tiles (r spilled to DRAM
    and re-loaded with partition-replicating cast DMAs).
  - pooling is a strided DVE reduce over xT (no PE work).
  - bo is added during the y evacuation (DVE tensor_tensor add).
"""

import os

import numpy as np

B, T, N, D = 16, 12, 1024, 512
H, HD, C = 8, 64, 128
NCORES = 8
BS = B // NCORES          # batch per core
NBT = BS * T              # (b, t) slices per core
MT = N // 128             # m-tiles per (b, t) slice  = 8
CI = D // 128             # contraction chunks        = 4
POOL = N // C             # pooling factor            = 8


def build_kernel(nc, n_bt=NBT):
    """Emit the full per-core kernel graph into `nc` (a bacc.Bacc)."""
    import concourse.bass as bass
    import concourse.tile as tile
    from concourse import mybir

    f32 = mybir.dt.float32
    bf16 = mybir.dt.bfloat16
    AF = mybir.ActivationFunctionType
    ALU = mybir.AluOpType

    M = n_bt * N

    x_in = nc.dram_tensor("x", [BS, T, N, D], f32, kind="ExternalInput").ap()
    Wq_in = nc.dram_tensor("Wq", [D, D], f32, kind="ExternalInput").ap()
    bq_in = nc.dram_tensor("bq", [D], f32, kind="ExternalInput").ap()
    Wk_in = nc.dram_tensor("Wk", [D, D], f32, kind="ExternalInput").ap()
    bk_in = nc.dram_tensor("bk", [D], f32, kind="ExternalInput").ap()
    Wv_in = nc.dram_tensor("Wv", [D, D], f32, kind="ExternalInput").ap()
    bv_in = nc.dram_tensor("bv", [D], f32, kind="ExternalInput").ap()
    Wo_in = nc.dram_tensor("Wo", [D, D], f32, kind="ExternalInput").ap()
    bo_in = nc.dram_tensor("bo", [D], f32, kind="ExternalInput").ap()
    adp_in = nc.dram_tensor("adp_pos", [N, C], f32, kind="ExternalInput").ap()
    y_out = nc.dram_tensor("out", [BS, T, N, D], f32, kind="ExternalOutput").ap()

    x_flat = x_in.rearrange("b t n d -> (b t n) d")
    y_flat = y_out.rearrange("b t n d -> (b t n) d")

    with tile.TileContext(nc) as tc:
        with (
            tc.tile_pool(name="const", bufs=1) as const_pool,
            tc.tile_pool(name="dram", bufs=1, space="DRAM") as dram_pool,
            tc.tile_pool(name="xt", bufs=3) as xt_pool,
            tc.tile_pool(name="qt", bufs=2) as qt_pool,
            tc.tile_pool(name="pooled", bufs=2) as pooled_pool,
            tc.tile_pool(name="smx", bufs=2) as smx_pool,
            tc.tile_pool(name="outt", bufs=2) as outt_pool,
            tc.tile_pool(name="ysb", bufs=2) as y_pool,
            tc.tile_pool(name="psA", bufs=2, space="PSUM") as psA,
            tc.tile_pool(name="psB", bufs=2, space="PSUM") as psB,
        ):
            # bf16 copy of x in DRAM feeding the xbar transpose loads.
            # The first two slices' casts are issued before the weight
            # loads so the slice-0 transposes can start ASAP.
            x16 = dram_pool.tile([M, D], bf16, name="x16")

            # ---------------- constants / weights preload ----------------
            # W* layout: [128, ci*512 + dout] = W[ci*128 + p, dout]  (bf16)
            # Interleaved with the first two slices' x casts so the
            # critical prologue chain (x16 -> xT -> q-proj) starts ASAP.
            w_sb = {}

            def load_w(nm, w_ap):
                w_t = const_pool.tile([128, CI * D], bf16, name=f"W{nm}_sb")
                nc.gpsimd.dma_start(
                    out=w_t[:].rearrange("p (ci dout) -> p ci dout", ci=CI),
                    in_=w_ap.rearrange("(ci p) dout -> p ci dout", p=128),
                )
                w_sb[nm] = w_t

            for ci in range(CI):
                nc.gpsimd.dma_start(
                    out=x16[0:N, ci * 128:(ci + 1) * 128],
                    in_=x_flat[0:N, ci * 128:(ci + 1) * 128],
                )
            load_w("q", Wq_in)
            if n_bt > 1:
                for ci in range(CI):
                    nc.gpsimd.dma_start(
                        out=x16[N:2 * N, ci * 128:(ci + 1) * 128],
                        in_=x_flat[N:2 * N, ci * 128:(ci + 1) * 128],
                    )
            load_w("k", Wk_in)
            load_w("v", Wv_in)
            load_w("o", Wo_in)

            # per-partition bias tiles [128, dt] for the transposed q/k evacs
            bq_sb = const_pool.tile([128, CI], f32, name="bq_sb")
            nc.sync.dma_start(out=bq_sb[:], in_=bq_in.rearrange("(dt p) -> p dt", p=128))
            bk_sb = const_pool.tile([128, CI], f32, name="bk_sb")
            nc.sync.dma_start(out=bk_sb[:], in_=bk_in.rearrange("(dt p) -> p dt", p=128))

            # bv*8 row (vp psum holds 8*(xp_sum/8 @ Wv + bv); evac scales 1/8)
            bv_row = const_pool.tile([1, D], f32, name="bv_row")
            nc.sync.dma_start(out=bv_row[:], in_=bv_in.unsqueeze(0))
            bv8_row = const_pool.tile([1, D], bf16, name="bv8_row")
            nc.scalar.activation(bv8_row[:], bv_row[:], AF.Copy, scale=float(POOL))

            bo_row = const_pool.tile([1, D], f32, name="bo_row")
            nc.sync.dma_start(out=bo_row[:], in_=bo_in.unsqueeze(0))

            ones_k1 = const_pool.tile([1, 128], bf16, name="ones_k1")
            nc.vector.memset(ones_k1[:], 1.0)
            ones_f32 = const_pool.tile([1, 128], f32, name="ones_f32")
            nc.vector.memset(ones_f32[:], 1.0)

            # bo broadcast [128, 2*D] (two copies side by side, for the
            # two-mt y evacuation adds)
            bo_bc2 = const_pool.tile([128, 2 * D], f32, name="bo_bc2")
            ps_bo = psA.tile([128, 2 * D], f32, name="ps_bo", tag="A", bufs=2)
            for half in range(2):
                nc.tensor.matmul(
                    ps_bo[:, half * D:(half + 1) * D], ones_f32[:], bo_row[:],
                    start=True, stop=True,
                )
            nc.vector.tensor_copy(bo_bc2[:], ps_bo[:])

            # e_all: [128, 8*H]; slice h = cols [8h, 8h+8), ones in col h.
            # sums matmul lhsT -> head h's denominator lands in psum row h.
            e_all = const_pool.tile([128, 8 * H], bf16, name="e_all")
            nc.vector.memset(e_all[:], 0.0)
            for h in range(H):
                nc.vector.memset(e_all[:, 8 * h + h: 8 * h + h + 1], 1.0)

            # eadpT [c=128, n=N] = exp(adp_pos)^T  (bf16), built via natural
            # exp -> DRAM spill -> one xbar transpose.
            adp_f = const_pool.tile([128, MT * C], f32, name="adp_f")
            nc.sync.dma_start(
                out=adp_f[:].rearrange("p (nt c) -> p nt c", nt=MT),
                in_=adp_in.rearrange("(nt p) c -> p nt c", p=128),
            )
            eadp_nat = const_pool.tile([128, MT * C], bf16, name="eadp_nat")
            nc.scalar.activation(eadp_nat[:], adp_f[:], AF.Exp)
            eadp_dr = dram_pool.tile([N, C], bf16, name="eadp_dr")
            nc.sync.dma_start(
                out=eadp_dr[:].rearrange("(nt p) c -> p nt c", p=128),
                in_=eadp_nat[:].rearrange("p (nt c) -> p nt c", nt=MT),
            )
            eadpT = const_pool.tile([128, N], bf16, name="eadpT")
            nc.sync.dma_start(out=eadpT[:], in_=eadp_dr[:], transpose=True)

            # ---------------- per-slice emission helpers ----------------

            def stage_inputs(bt, cast=True):
                """x16 <- cast DMA (DRAM->DRAM, per column chunk so each
                xbar transpose can start as soon as its chunk lands);
                xT <- 4 xbar transposes."""
                r0 = bt * N
                xT = xt_pool.tile([128, CI * N], bf16, name="xT", tag="xT")
                for ci in range(CI):
                    if cast:
                        nc.gpsimd.dma_start(
                            out=x16[r0:r0 + N, ci * 128:(ci + 1) * 128],
                            in_=x_flat[r0:r0 + N, ci * 128:(ci + 1) * 128],
                        )
                    nc.sync.dma_start(
                        out=xT[:, ci * N:(ci + 1) * N],
                        in_=x16[r0:r0 + N, ci * 128:(ci + 1) * 128],
                        transpose=True,
                    )
                return xT

            def pool_reduce(xT):
                """Strided DVE pooling over xT -> xpT, emitted at the START
                of the previous slice's head loop so these DVE ops precede
                that slice's u-mults/reciprocal in the DVE queue."""
                xpT = pooled_pool.tile([128, CI * C], bf16, name="xpT")
                with nc.allow_low_precision("pool sums, bf16 ok"):
                    for ci in range(CI):
                        nc.vector.reduce_sum(
                            xpT[:, ci * C:(ci + 1) * C],
                            xT[:, ci * N:(ci + 1) * N]
                            .rearrange("p (j i) -> p j i", i=POOL),
                            axis=mybir.AxisListType.X,
                        )
                return xpT

            def proj_chunks(xT, xpT, kv_ps_fn):
                """Generator of PE work for one slice's projections; yields
                between chunks so the caller interleaves it into the softmax
                head loop. Final yield returns (qT, kpT, vp)."""
                qT = qt_pool.tile([128, CI * N], bf16, name="qT")
                for dt in range(CI):
                    ps_q = psA.tile([128, N], f32, name="ps_q", tag="A", bufs=2)
                    # ci outer / mc inner: consecutive matmuls share the
                    # stationary tile, letting LDWEIGHTS be skipped/hidden
                    for ci in range(CI):
                        for mc in range(2):
                            nc.tensor.matmul(
                                ps_q[:, mc * 512:(mc + 1) * 512],
                                w_sb["q"][:, ci * D + dt * 128: ci * D + dt * 128 + 128],
                                xT[:, ci * N + mc * 512: ci * N + mc * 512 + 512],
                                start=(ci == 0),
                                stop=(ci == CI - 1),
                            )
                    nc.scalar.activation(
                        qT[:, dt * N:(dt + 1) * N], ps_q[:],
                        AF.Identity, bias=bq_sb[:, dt:dt + 1],
                    )
                    yield None

                # kp (transposed) + vp (natural) share one psum tile,
                # allocated by the caller BEFORE ps_s so the psB ring pairs
                # the attn@v tiles with this early-freed tile instead of
                # ps_s (whose release needs the reciprocal).
                ps_kv = kv_ps_fn()
                kpT = pooled_pool.tile([128, CI * C], bf16, name="kpT")
                vp = pooled_pool.tile([128, D], bf16, name="vp")
                for dt in range(CI):
                    for ci in range(CI):
                        nc.tensor.matmul(
                            ps_kv[:, dt * 128:(dt + 1) * 128],
                            w_sb["k"][:, ci * D + dt * 128: ci * D + dt * 128 + 128],
                            xpT[:, ci * C:(ci + 1) * C],
                            start=(ci == 0),
                            stop=(ci == CI - 1),
                        )
                yield None
                for ci in range(CI):
                    nc.tensor.matmul(
                        ps_kv[:, 512:],
                        xpT[:, ci * C:(ci + 1) * C],
                        w_sb["v"][:, ci * D:(ci + 1) * D],
                        start=(ci == 0),
                        stop=False,
                    )
                nc.tensor.matmul(
                    ps_kv[:, 512:], ones_k1[:], bv8_row[:], start=False, stop=True
                )
                for dt in range(CI):
                    nc.scalar.activation(
                        kpT[:, dt * 128:(dt + 1) * 128],
                        ps_kv[:, dt * 128:(dt + 1) * 128],
                        AF.Identity, scale=1.0 / POOL, bias=bk_sb[:, dt:dt + 1],
                    )
                nc.scalar.activation(vp[:], ps_kv[:, 512:], AF.Copy, scale=1.0 / POOL)
                yield (qT, kpT, vp)

            def head_loop(qkv, proj_gen):
                """Scores/exp/u/sums for all 8 heads of one slice, with the
                NEXT slice's q-projection chunks interleaved into the PE
                gaps. Returns (u_all, ps_s, next_qT)."""
                qT, kpT, vp = qkv
                u_all = smx_pool.tile([128, H * N], bf16, name="u_all", bufs=1)
                if proj_gen is not None:
                    kv_holder["t"] = psB.tile([128, N], f32, name="ps_kv",
                                              tag="B", bufs=2)
                ps_s = psB.tile([128, N], f32, name="ps_s", tag="B", bufs=2)
                next_qkv = None
                pending_sums = []

                def drain_one_proj():
                    nonlocal next_qkv
                    if proj_gen is not None:
                        try:
                            r = next(proj_gen)
                            if r is not None:
                                next_qkv = r
                        except StopIteration:
                            pass

                for h in range(H):
                    ph = (h % 2) * 64
                    dt = h // 2
                    ps_sc = psA.tile([128, N], f32, name="ps_sc", tag="A", bufs=2)
                    for half in range(2):
                        nc.tensor.matmul(
                            ps_sc[:, half * 512:(half + 1) * 512],
                            kpT[ph:ph + 64, dt * 128:(dt + 1) * 128],
                            qT[ph:ph + 64, dt * N + half * 512: dt * N + (half + 1) * 512],
                            start=True, stop=True,
                        )
                    exp_sb = smx_pool.tile([128, N], bf16, name="exp_sb", bufs=2)
                    nc.scalar.activation(exp_sb[:], ps_sc[:], AF.Exp, scale=1.0 / 8.0)
                    nc.vector.tensor_tensor(
                        u_all[:, h * N:(h + 1) * N], exp_sb[:], eadpT[:], op=ALU.mult
                    )
                    # interleave projection work into the exp/u latency gap
                    drain_one_proj()
                    if pending_sums:
                        hp = pending_sums.pop(0)
                        for half in range(2):
                            nc.tensor.matmul(
                                ps_s[0:8, half * 512:(half + 1) * 512],
                                e_all[:, 8 * hp: 8 * hp + 8],
                                u_all[:, hp * N + half * 512: hp * N + (half + 1) * 512],
                                start=(hp == 0), stop=(hp == H - 1),
                            )
                    pending_sums.append(h)
                while pending_sums:
                    hp = pending_sums.pop(0)
                    for half in range(2):
                        nc.tensor.matmul(
                            ps_s[0:8, half * 512:(half + 1) * 512],
                            e_all[:, 8 * hp: 8 * hp + 8],
                            u_all[:, hp * N + half * 512: hp * N + (half + 1) * 512],
                            start=(hp == 0), stop=(hp == H - 1),
                        )
                # drain any remaining projection chunks
                for _ in range(8):
                    drain_one_proj()
                return u_all, ps_s, next_qkv

            def norm_and_attn(u_all, ps_s, vp, bt):
                """reciprocal + r broadcast + attn@v + scaled evac -> outT."""
                r_f32 = smx_pool.tile([8, N], f32, name="r_f32", bufs=2)
                nc.vector.reciprocal_approx_fast(r_f32[:], ps_s[0:8, :])
                r_dr = dram_pool.tile([8, N], f32, name="r_dr", tag="r_dr", bufs=2)
                nc.gpsimd.dma_start(out=r_dr[:], in_=r_f32[:])
                rbc = []
                for pp in range(4):
                    t = smx_pool.tile([128, N], bf16, name=f"rbc{pp}", bufs=2)
                    for j in range(2):
                        nc.gpsimd.dma_start(
                            out=t[j * 64:(j + 1) * 64, :],
                            in_=r_dr[2 * pp + j: 2 * pp + j + 1, :].broadcast_to((64, N)),
                        )
                    rbc.append(t)

                outT = outt_pool.tile([128, CI * N], bf16, name="outT")
                for pp in range(4):
                    ps_o = psB.tile([128, N], f32, name="ps_o", tag="B", bufs=2)
                    for j in range(2):
                        h = 2 * pp + j
                        for half in range(2):
                            nc.tensor.matmul(
                                ps_o[j * 64:(j + 1) * 64, half * 512:(half + 1) * 512],
                                vp[:, h * 64:(h + 1) * 64],
                                u_all[:, h * N + half * 512: h * N + (half + 1) * 512],
                                start=True, stop=True,
                                tile_position=(0, j * 64),
                            )
                    nc.vector.tensor_tensor(
                        outT[:, pp * N:(pp + 1) * N], ps_o[:], rbc[pp][:], op=ALU.mult
                    )
                return outT

            def out_proj(outT, bt):
                """y = outT^T @ Wo + bo, two m-tiles per psum tile."""
                r0 = bt * N
                for mp in range(4):
                    ps_y = psA.tile([128, N], f32, name="ps_y", tag="A", bufs=2)
                    for half in range(2):
                        mt = 2 * mp + half
                        for pp in range(4):
                            nc.tensor.matmul(
                                ps_y[:, half * 512:(half + 1) * 512],
                                outT[:, pp * N + mt * 128: pp * N + (mt + 1) * 128],
                                w_sb["o"][:, pp * D:(pp + 1) * D],
                                start=(pp == 0),
                                stop=(pp == 3),
                            )
                    y_sb = y_pool.tile([128, N], f32, name="y_sb")
                    nc.vector.tensor_tensor(y_sb[:], ps_y[:], bo_bc2[:], op=ALU.add)
                    nc.gpsimd.dma_start(
                        out=y_flat[r0 + 2 * mp * 128: r0 + (2 * mp + 2) * 128, :]
                        .rearrange("(two p) d -> p two d", p=128),
                        in_=y_sb[:].rearrange("p (two d) -> p two d", two=2),
                    )

            # ---------------- main loop ----------------
            kv_holder = {}
            xTs = [stage_inputs(0, cast=False)]
            if n_bt > 1:
                xTs.append(stage_inputs(1, cast=False))
            # prologue: slice-0 projections up front
            kv_holder["t"] = psB.tile([128, N], f32, name="ps_kv",
                                      tag="B", bufs=2)
            qkv = None
            for r in proj_chunks(xTs[0], pool_reduce(xTs[0]),
                                 lambda: kv_holder.pop("t")):
                if r is not None:
                    qkv = r

            # out_proj for slice bt is emitted one iteration late (after
            # head_loop(bt+1)) so its y-adds queue behind the next slice's
            # u-mults on the DVE, instead of starving the PE's sums.
            pending = None
            for bt in range(n_bt):
                if bt + 2 < n_bt:
                    xTs.append(stage_inputs(bt + 2))
                gen = None
                if bt + 1 < n_bt:
                    gen = proj_chunks(xTs[bt + 1], pool_reduce(xTs[bt + 1]),
                                      lambda: kv_holder.pop("t"))
                u_all, ps_s, next_qkv = head_loop(qkv, gen)
                _, _, vp = qkv
                outT = norm_and_attn(u_all, ps_s, vp, bt)
                if pending is not None:
                    out_proj(*pending)
                pending = (outT, bt)
                qkv = next_qkv
            out_proj(*pending)

    return nc


_COMPILED = {}


def _get_compiled(n_bt=NBT):
    if n_bt not in _COMPILED:
        from concourse import bacc

        nc = bacc.Bacc("TRN2", target_bir_lowering=False, debug=False,
                       num_devices=NCORES)
        build_kernel(nc, n_bt)
        nc.compile()
        _COMPILED[n_bt] = nc
    return _COMPILED[n_bt]


def kernel(**inputs):
    """Full-input entry point: shards over batch across 8 cores."""
    os.environ.setdefault("JAX_PLATFORMS", "axon,cpu")
    os.environ.setdefault("NEURON_RT_RESET_CORES", "1")
    from concourse.bass_utils import run_bass_kernel_spmd

    nc = _get_compiled()

    x = np.ascontiguousarray(inputs["x"], dtype=np.float32)
    params = {
        k: np.ascontiguousarray(inputs[k], dtype=np.float32)
        for k in ("Wq", "bq", "Wk", "bk", "Wv", "bv", "Wo", "bo", "adp_pos")
    }
    in_maps = []
    for core in range(NCORES):
        m = {"x": x[core * BS:(core + 1) * BS]}
        m.update(params)
        in_maps.append(m)

    res = run_bass_kernel_spmd(nc, in_maps, core_ids=list(range(NCORES)))
    out = np.concatenate([res.results[i]["out"] for i in range(NCORES)], axis=0)
    return out


if __name__ == "__main__":
    import jax

    jax.config.update("jax_platforms", "cpu")
    import reference

    inputs = reference.setup_inputs()
    inputs = {k: np.asarray(v) for k, v in inputs.items()}
    expected = np.asarray(reference.reference(**inputs))
    actual = kernel(**inputs)
    err = np.linalg.norm(actual - expected) / np.linalg.norm(expected)
    print("Relative error:", err)
